# revision 26
# baseline (speedup 1.0000x reference)
"""Trainium2 Bass kernel for nn_Attention_st_2010044694918.

Reference computation (per sample b of B=256):
    q = x[b, :64]                 # [64, 768]
    k = v = x[b, 64:]             # [256, 768]
    S = q @ k.T * 64**-0.5        # [64, 256]
    P = softmax(S, axis=-1)
    out = P @ v                   # [64, 768]
    s = out.T.reshape(64, 768)    # channel-major scramble
    y = s @ proj_w.T + proj_b     # [64, 768]
    result[b] = concat([y, k])    # [320, 768]

Device strategy (pure data parallel, 32 samples / core on 8 cores):
  - host ships x[b].T (for the QK^T matmul, contraction over channels) and
    k natural (for the PV matmul, contraction over keys) plus proj_w.T; all
    three are pre-blocked on the host into the exact [128, free] SBUF layout
    so every input DMA is a single fully-contiguous transfer.
  - the scramble is folded into the final matmul: with OUT2 = [out ; out
    shifted left one column], row-pair r=(2c, 2c+1) of the scramble is the
    strided view OUT2[:, 2c::12][:, :64], and y = sum_c over 6 accumulating
    matmuls against contiguous 128-row slabs of proj_w.T.
  - softmax normalization is folded into the PSUM eviction of out (per-
    partition scalar multiply); bias is added during the PSUM eviction of y.
  - per-sample work is emitted as a software pipeline (skewed stages) so the
    tensor engine sees a dense back-to-back matmul stream (HAM stays warm).
  - the k-passthrough half of the output never touches the device; the host
    assembles it.
"""

import numpy as np

import concourse.bass as bass
import concourse.tile as tile
from concourse import bacc
from concourse import mybir
from concourse.bass_utils import run_bass_kernel_spmd
from concourse.masks import make_identity

B, N, C = 256, 320, 768
LZ = 64          # query tokens
LK = N - LZ      # key tokens (256)
NCORES = 8
BS = B // NCORES  # samples per core
SCALE = (C // 12) ** -0.5  # head_dim**-0.5 = 0.125

F32 = mybir.dt.float32
MM_DT = mybir.dt.float16  # ~tf32-precision inputs, 2-byte DMA + fast weight loads


def build_nc(bs: int = BS):
    assert bs % 2 == 0
    nc = bacc.Bacc("TRN2", target_bir_lowering=False)
    # pre-blocked inputs: [.., 128, free] matching SBUF tiles exactly
    xt_d = nc.dram_tensor("xtb", [bs // 2, 128, 12 * N], MM_DT, kind="ExternalInput")
    kn_d = nc.dram_tensor("knb", [bs // 2, 128, 4 * C], MM_DT, kind="ExternalInput")
    pwt_d = nc.dram_tensor("pwtb", [128, 6 * C], MM_DT, kind="ExternalInput")
    b64_d = nc.dram_tensor("bias64", [128, C], F32, kind="ExternalInput")
    y_d = nc.dram_tensor("y", [bs * LZ, C], MM_DT, kind="ExternalOutput")

    with tile.TileContext(nc) as tc:
        with (
            tc.tile_pool(name="consts", bufs=1) as consts,
            tc.tile_pool(name="xt", bufs=3) as xt_pool,
            tc.tile_pool(name="kn", bufs=5) as kn_pool,
            tc.tile_pool(name="exps", bufs=5) as exps_pool,
            tc.tile_pool(name="recip", bufs=8) as recip_pool,
            tc.tile_pool(name="pt", bufs=3) as pt_pool,
            tc.tile_pool(name="out2", bufs=4) as out2_pool,
            tc.tile_pool(name="ysb", bufs=3) as y_pool,
            tc.tile_pool(name="ps_s", bufs=2, space="PSUM") as psum_s,
            tc.tile_pool(name="ps_pt", bufs=2, space="PSUM") as psum_pt,
            tc.tile_pool(name="ps_o", bufs=1, space="PSUM") as psum_o,
            tc.tile_pool(name="ps_y", bufs=1, space="PSUM") as psum_y,
        ):
            ident = consts.tile([LZ, LZ], MM_DT)
            make_identity(nc, ident[:])
            pwt_t = consts.tile([128, 6 * C], MM_DT)
            nc.scalar.dma_start(pwt_t[:], pwt_d[:])
            b64_t = consts.tile([128, C], F32)
            nc.scalar.dma_start(b64_t[:], b64_d[:])

            st = [dict() for _ in range(bs)]  # per-sample tiles

            def stage_load_xt(b):
                # one ~1MB fully-contiguous DMA per sample PAIR
                if b % 2 == 1:
                    return
                xt_t = xt_pool.tile([128, 12 * N], MM_DT, tag="xt")
                nc.sync.dma_start(xt_t[:], xt_d[b // 2])
                st[b]["xt"] = xt_t
                st[b + 1]["xt"] = xt_t

            def stage_load_kn(b):
                if b % 2 == 1:
                    return
                kn_t = kn_pool.tile([128, 4 * C], MM_DT, tag="kn")
                nc.sync.dma_start(kn_t[:], kn_d[b // 2])
                st[b]["kn"] = kn_t
                st[b + 1]["kn"] = kn_t

            def stage_s(b):
                # S = q @ k.T, contraction over channels in 6 chunks of 128
                xt_t = st[b].pop("xt")
                xo = (b % 2) * 6 * N
                ps_s = psum_s.tile([LZ, LK], F32, tag="s")
                for cc in range(6):
                    nc.tensor.matmul(
                        ps_s[:],
                        xt_t[:, xo + cc * N : xo + cc * N + LZ],
                        xt_t[:, xo + cc * N + LZ : xo + (cc + 1) * N],
                        start=(cc == 0),
                        stop=(cc == 5),
                    )
                st[b]["ps_s"] = ps_s

            def stage_exp(b):
                # fp16 exp needs max subtraction: exp(scale*S - scale*max(S));
                # the shift cancels exactly in P = exps * (1/rowsum)
                ps_s = st[b].pop("ps_s")
                exps = exps_pool.tile([LZ, LK], MM_DT, tag="exps")
                mxneg = recip_pool.tile([LZ, 1], F32, tag="mxneg")
                rowsum = recip_pool.tile([LZ, 1], F32, tag="rowsum")
                recip = recip_pool.tile([LZ, 1], F32, tag="recip")
                nc.vector.tensor_reduce(
                    mxneg[:], ps_s[:], axis=mybir.AxisListType.X,
                    op=mybir.AluOpType.max, negate=True,
                )
                nc.scalar.activation(
                    exps[:],
                    ps_s[:],
                    mybir.ActivationFunctionType.Exp,
                    bias=mxneg[:],
                    accum_out=rowsum[:],
                )
                nc.vector.reciprocal(recip[:], rowsum[:])
                st[b]["exps"] = exps
                st[b]["recip"] = recip

            def stage_pt(b):
                # P^T via tensor-engine transpose (two 64x128 -> 128x64)
                exps = st[b].pop("exps")
                ps_pt = psum_pt.tile([128, 2 * LZ], MM_DT, tag="pt")
                nc.tensor.transpose(ps_pt[:, 0:LZ], exps[:, 0:128], ident[:])
                nc.tensor.transpose(ps_pt[:, LZ : 2 * LZ], exps[:, 128:256], ident[:])
                pt_sb = pt_pool.tile([128, 2 * LZ], MM_DT, tag="pt_sb")
                nc.vector.tensor_copy(pt_sb[:], ps_pt[:])
                st[b]["pt"] = pt_sb

            def stage_av(b):
                # out = P @ k (unnormalized), contraction over 256 keys
                pt_sb = st[b].pop("pt")
                kn_t = st[b].pop("kn")
                ko = (b % 2) * 2 * C
                ps_o = psum_o.tile([LZ, C], F32, tag="o")
                for h0, h1 in ((0, 512), (512, C)):
                    for j in (0, 1):
                        nc.tensor.matmul(
                            ps_o[:, h0:h1],
                            pt_sb[:, j * LZ : (j + 1) * LZ],
                            kn_t[:, ko + j * C + h0 : ko + j * C + h1],
                            start=(j == 0),
                            stop=(j == 1),
                        )
                st[b]["ps_o"] = ps_o

            def stage_norm(b):
                # OUT2 = [out (normalized) ; out shifted left one column];
                # two samples side by side in the free dim of one pair tile
                ps_o = st[b].pop("ps_o")
                recip = st[b].pop("recip")
                if b % 2 == 0:
                    out2 = out2_pool.tile([128, 2 * C], MM_DT, tag="out2")
                    st[b]["out2"] = out2
                else:
                    out2 = st[b - 1]["out2"]
                co = (b % 2) * C
                nc.vector.tensor_scalar_mul(out2[0:LZ, co : co + C], ps_o[:], recip[:])
                # shifted half normalized straight from PSUM on ACT (parallel
                # with the DVE op above, both read ps_o)
                nc.scalar.activation(
                    out2[LZ:128, co : co + C - 1],
                    ps_o[:, 1:C],
                    mybir.ActivationFunctionType.Copy,
                    scale=recip[:],
                )

            def stage_proj(b):
                # y = scramble(out) @ proj_w.T for a PAIR of samples: the
                # weight slabs are shared, so sample b fills array columns
                # 0:64 and sample b+1 columns 64:128 (M=128 per matmul)
                if b % 2 == 0:
                    return
                out2 = st[b - 1].pop("out2")
                ps_y = psum_y.tile([128, C], F32, tag="ps_y")
                o2r = out2[:].rearrange("p (g i r) -> p r g i", r=12, g=2)
                for h0, h1 in ((0, 512), (512, C)):
                    for cc2 in range(6):
                        nc.tensor.matmul(
                            ps_y[:, h0:h1],
                            o2r[:, 2 * cc2],
                            pwt_t[:, cc2 * C + h0 : cc2 * C + h1],
                            start=(cc2 == 0),
                            stop=(cc2 == 5),
                        )
                st[b]["ps_y"] = ps_y

            def stage_y(b):
                # bias add during PSUM eviction; ship pairs of samples
                if b % 2 == 0:
                    return
                ps_y = st[b].pop("ps_y")
                ysb = y_pool.tile([128, C], MM_DT, tag="ysb")
                nc.vector.tensor_add(ysb[:], ps_y[:], b64_t[:])
                nc.sync.dma_start(y_d[(b - 1) * LZ : (b + 1) * LZ, :], ysb[:])

            stages = [
                (stage_load_xt, 0),
                (stage_load_kn, 1),
                (stage_s, 2),
                (stage_exp, 3),
                (stage_pt, 6),
                (stage_av, 7),
                (stage_norm, 8),
                (stage_proj, 11),
                (stage_y, 12),
            ]
            max_skew = max(sk for _, sk in stages)
            for i in range(bs + max_skew):
                for fn, sk in stages:
                    b = i - sk
                    if 0 <= b < bs:
                        fn(b)

    nc.compile()
    return nc


_NC_CACHE = {}


def _get_nc(bs: int = BS):
    if bs not in _NC_CACHE:
        _NC_CACHE[bs] = build_nc(bs)
    return _NC_CACHE[bs]


def _host_prep(x, proj_w, proj_b):
    """Pre-block inputs into the exact SBUF layouts (contiguous DMAs)."""
    x = np.asarray(x, dtype=np.float32)
    proj_w = np.asarray(proj_w, dtype=np.float32)
    proj_b = np.asarray(proj_b, dtype=np.float32)

    mmnp = mybir.dt.np(MM_DT)
    # xtb[b, p, cc*N + t] = x[b, t, cc*128 + p]; the softmax scale is folded
    # into the query columns (t < LZ) so S arrives pre-scaled
    xtb = x.reshape(B, N, 6, 128).transpose(0, 3, 2, 1).reshape(B, 128, 6 * N)
    xtb = np.ascontiguousarray(xtb, dtype=np.float32).reshape(B, 128, 6, N)
    xtb[:, :, :, :LZ] *= SCALE
    xtb = xtb.reshape(B // 2, 2, 128, 6 * N).transpose(0, 2, 1, 3)
    xtb = np.ascontiguousarray(xtb.reshape(B // 2, 128, 12 * N), dtype=mmnp)
    # knb[b, p, j*C + c] = x[b, LZ + j*128 + p, c]
    knb = x[:, LZ:, :].reshape(B, 2, 128, C).transpose(0, 2, 1, 3).reshape(B, 128, 2 * C)
    knb = knb.reshape(B // 2, 2, 128, 2 * C).transpose(0, 2, 1, 3)
    knb = np.ascontiguousarray(knb.reshape(B // 2, 128, 4 * C), dtype=mmnp)
    # pwtb[p, cc*C + m] = proj_w.T[cc*128 + p, m] = proj_w[m, cc*128 + p]
    pwtb = np.ascontiguousarray(
        proj_w.T.reshape(6, 128, C).transpose(1, 0, 2).reshape(128, 6 * C),
        dtype=mmnp,
    )
    b64 = np.ascontiguousarray(np.broadcast_to(proj_b, (128, C)))
    return x, xtb, knb, pwtb, b64


def _run(x, proj_w, proj_b, **spmd_kwargs):
    x, xtb, knb, pwtb, b64 = _host_prep(x, proj_w, proj_b)

    nc = _get_nc()
    in_maps = [
        {
            "xtb": xtb[i * BS // 2 : (i + 1) * BS // 2],
            "knb": knb[i * BS // 2 : (i + 1) * BS // 2],
            "pwtb": pwtb,
            "bias64": b64,
        }
        for i in range(NCORES)
    ]
    res = run_bass_kernel_spmd(
        nc, in_maps, core_ids=list(range(NCORES)), **spmd_kwargs
    )

    out = np.empty((B, N, C), dtype=np.float32)
    out[:, LZ:, :] = x[:, LZ:, :]
    for i in range(NCORES):
        out[i * BS : (i + 1) * BS, :LZ, :] = res.results[i]["y"].reshape(BS, LZ, C)
    return out, res


def kernel(x, proj_w, proj_b):
    out, _ = _run(x, proj_w, proj_b)
    return out


# revision 28
# speedup vs baseline: 1.0369x; 1.0369x over previous
"""Trainium2 Bass kernel for nn_Attention_st_2010044694918.

Reference computation (per sample b of B=256):
    q = x[b, :64]                 # [64, 768]
    k = v = x[b, 64:]             # [256, 768]
    S = q @ k.T * 64**-0.5        # [64, 256]
    P = softmax(S, axis=-1)
    out = P @ v                   # [64, 768]
    s = out.T.reshape(64, 768)    # channel-major scramble
    y = s @ proj_w.T + proj_b     # [64, 768]
    result[b] = concat([y, k])    # [320, 768]

Device strategy (pure data parallel, 32 samples / core on 8 cores):
  - host ships x[b].T (for the QK^T matmul, contraction over channels) and
    k natural (for the PV matmul, contraction over keys) plus proj_w.T; all
    three are pre-blocked on the host into the exact [128, free] SBUF layout
    so every input DMA is a single fully-contiguous transfer.
  - the scramble is folded into the final matmul: with OUT2 = [out ; out
    shifted left one column], row-pair r=(2c, 2c+1) of the scramble is the
    strided view OUT2[:, 2c::12][:, :64], and y = sum_c over 6 accumulating
    matmuls against contiguous 128-row slabs of proj_w.T.
  - softmax normalization is folded into the PSUM eviction of out (per-
    partition scalar multiply); bias is added during the PSUM eviction of y.
  - per-sample work is emitted as a software pipeline (skewed stages) so the
    tensor engine sees a dense back-to-back matmul stream (HAM stays warm).
  - the k-passthrough half of the output never touches the device; the host
    assembles it.
"""

import numpy as np

import concourse.bass as bass
import concourse.tile as tile
from concourse import bacc
from concourse import mybir
from concourse.bass_utils import run_bass_kernel_spmd
from concourse.masks import make_identity

B, N, C = 256, 320, 768
LZ = 64          # query tokens
LK = N - LZ      # key tokens (256)
NCORES = 8
BS = B // NCORES  # samples per core
SCALE = (C // 12) ** -0.5  # head_dim**-0.5 = 0.125

F32 = mybir.dt.float32
MM_DT = mybir.dt.float16  # ~tf32-precision inputs, 2-byte DMA + fast weight loads


def build_nc(bs: int = BS):
    assert bs % 2 == 0
    nc = bacc.Bacc("TRN2", target_bir_lowering=False)
    # pre-blocked inputs: [.., 128, free] matching SBUF tiles exactly
    xt_d = nc.dram_tensor("xtb", [bs, 128, 6 * N], MM_DT, kind="ExternalInput")
    kn_d = nc.dram_tensor("knb", [bs, 128, 2 * C], MM_DT, kind="ExternalInput")
    pwt_d = nc.dram_tensor("pwtb", [128, 6 * C], MM_DT, kind="ExternalInput")
    b64_d = nc.dram_tensor("bias64", [128, C], F32, kind="ExternalInput")
    y_d = nc.dram_tensor("y", [bs * LZ, C], MM_DT, kind="ExternalOutput")

    with tile.TileContext(nc) as tc:
        with (
            tc.tile_pool(name="consts", bufs=1) as consts,
            tc.tile_pool(name="xt", bufs=5) as xt_pool,
            tc.tile_pool(name="kn", bufs=9) as kn_pool,
            tc.tile_pool(name="exps", bufs=5) as exps_pool,
            tc.tile_pool(name="recip", bufs=8) as recip_pool,
            tc.tile_pool(name="pt", bufs=3) as pt_pool,
            tc.tile_pool(name="out2", bufs=4) as out2_pool,
            tc.tile_pool(name="ysb", bufs=3) as y_pool,
            tc.tile_pool(name="ps_s", bufs=2, space="PSUM") as psum_s,
            tc.tile_pool(name="ps_pt", bufs=2, space="PSUM") as psum_pt,
            tc.tile_pool(name="ps_o", bufs=1, space="PSUM") as psum_o,
            tc.tile_pool(name="ps_y", bufs=1, space="PSUM") as psum_y,
        ):
            ident = consts.tile([LZ, LZ], MM_DT)
            make_identity(nc, ident[:])
            pwt_t = consts.tile([128, 6 * C], MM_DT)
            nc.scalar.dma_start(pwt_t[:], pwt_d[:])
            b64_t = consts.tile([128, C], F32)
            nc.scalar.dma_start(b64_t[:], b64_d[:])

            st = [dict() for _ in range(bs)]  # per-sample tiles

            def stage_load_xt(b):
                xt_t = xt_pool.tile([128, 6 * N], MM_DT, tag="xt")
                nc.sync.dma_start(xt_t[:], xt_d[b])
                st[b]["xt"] = xt_t

            def stage_load_kn(b):
                kn_t = kn_pool.tile([128, 2 * C], MM_DT, tag="kn")
                nc.sync.dma_start(kn_t[:], kn_d[b])
                st[b]["kn"] = kn_t

            def stage_s(b):
                # S = q @ k.T, contraction over channels in 6 chunks of 128
                xt_t = st[b].pop("xt")
                ps_s = psum_s.tile([LZ, LK], F32, tag="s")
                for cc in range(6):
                    nc.tensor.matmul(
                        ps_s[:],
                        xt_t[:, cc * N : cc * N + LZ],
                        xt_t[:, cc * N + LZ : (cc + 1) * N],
                        start=(cc == 0),
                        stop=(cc == 5),
                    )
                st[b]["ps_s"] = ps_s

            def stage_exp(b):
                # fp16 exp needs max subtraction: exp(scale*S - scale*max(S));
                # the shift cancels exactly in P = exps * (1/rowsum)
                ps_s = st[b].pop("ps_s")
                exps = exps_pool.tile([LZ, LK], MM_DT, tag="exps")
                mxneg = recip_pool.tile([LZ, 1], F32, tag="mxneg")
                rowsum = recip_pool.tile([LZ, 1], F32, tag="rowsum")
                recip = recip_pool.tile([LZ, 1], F32, tag="recip")
                nc.vector.tensor_reduce(
                    mxneg[:], ps_s[:], axis=mybir.AxisListType.X,
                    op=mybir.AluOpType.max, negate=True,
                )
                nc.scalar.activation(
                    exps[:],
                    ps_s[:],
                    mybir.ActivationFunctionType.Exp,
                    bias=mxneg[:],
                    accum_out=rowsum[:],
                )
                nc.vector.reciprocal(recip[:], rowsum[:])
                st[b]["exps"] = exps
                st[b]["recip"] = recip

            def stage_pt(b):
                # P^T via tensor-engine transpose (two 64x128 -> 128x64)
                exps = st[b].pop("exps")
                ps_pt = psum_pt.tile([128, 2 * LZ], MM_DT, tag="pt")
                nc.tensor.transpose(ps_pt[:, 0:LZ], exps[:, 0:128], ident[:])
                nc.tensor.transpose(ps_pt[:, LZ : 2 * LZ], exps[:, 128:256], ident[:])
                pt_sb = pt_pool.tile([128, 2 * LZ], MM_DT, tag="pt_sb")
                nc.vector.tensor_copy(pt_sb[:], ps_pt[:])
                st[b]["pt"] = pt_sb

            def stage_av(b):
                # out = P @ k (unnormalized), contraction over 256 keys
                pt_sb = st[b].pop("pt")
                kn_t = st[b].pop("kn")
                ps_o = psum_o.tile([LZ, C], F32, tag="o")
                for h0, h1 in ((0, 512), (512, C)):
                    for j in (0, 1):
                        nc.tensor.matmul(
                            ps_o[:, h0:h1],
                            pt_sb[:, j * LZ : (j + 1) * LZ],
                            kn_t[:, j * C + h0 : j * C + h1],
                            start=(j == 0),
                            stop=(j == 1),
                        )
                st[b]["ps_o"] = ps_o

            def stage_norm(b):
                # OUT2 = [out (normalized) ; out shifted left one column];
                # two samples side by side in the free dim of one pair tile
                ps_o = st[b].pop("ps_o")
                recip = st[b].pop("recip")
                if b % 2 == 0:
                    out2 = out2_pool.tile([128, 2 * C], MM_DT, tag="out2")
                    st[b]["out2"] = out2
                else:
                    out2 = st[b - 1]["out2"]
                co = (b % 2) * C
                nc.vector.tensor_scalar_mul(out2[0:LZ, co : co + C], ps_o[:], recip[:])
                # shifted half normalized straight from PSUM on ACT (parallel
                # with the DVE op above, both read ps_o)
                nc.scalar.activation(
                    out2[LZ:128, co : co + C - 1],
                    ps_o[:, 1:C],
                    mybir.ActivationFunctionType.Copy,
                    scale=recip[:],
                )

            def stage_proj(b):
                # y = scramble(out) @ proj_w.T for a PAIR of samples: the
                # weight slabs are shared, so sample b fills array columns
                # 0:64 and sample b+1 columns 64:128 (M=128 per matmul)
                if b % 2 == 0:
                    return
                out2 = st[b - 1].pop("out2")
                ps_y = psum_y.tile([128, C], F32, tag="ps_y")
                o2r = out2[:].rearrange("p (g i r) -> p r g i", r=12, g=2)
                for h0, h1 in ((0, 512), (512, C)):
                    for cc2 in range(6):
                        nc.tensor.matmul(
                            ps_y[:, h0:h1],
                            o2r[:, 2 * cc2],
                            pwt_t[:, cc2 * C + h0 : cc2 * C + h1],
                            start=(cc2 == 0),
                            stop=(cc2 == 5),
                        )
                st[b]["ps_y"] = ps_y

            def stage_y(b):
                # bias add during PSUM eviction; ship pairs of samples
                if b % 2 == 0:
                    return
                ps_y = st[b].pop("ps_y")
                ysb = y_pool.tile([128, C], MM_DT, tag="ysb")
                nc.vector.tensor_add(ysb[:], ps_y[:], b64_t[:])
                nc.scalar.dma_start(y_d[(b - 1) * LZ : (b + 1) * LZ, :], ysb[:])

            stages = [
                (stage_load_xt, 0),
                (stage_load_kn, 1),
                (stage_s, 2),
                (stage_exp, 3),
                (stage_pt, 6),
                (stage_av, 7),
                (stage_norm, 8),
                (stage_proj, 11),
                (stage_y, 12),
            ]
            max_skew = max(sk for _, sk in stages)
            for i in range(bs + max_skew):
                for fn, sk in stages:
                    b = i - sk
                    if 0 <= b < bs:
                        fn(b)

    nc.compile()
    return nc


_NC_CACHE = {}


def _get_nc(bs: int = BS):
    if bs not in _NC_CACHE:
        _NC_CACHE[bs] = build_nc(bs)
    return _NC_CACHE[bs]


def _host_prep(x, proj_w, proj_b):
    """Pre-block inputs into the exact SBUF layouts (contiguous DMAs)."""
    x = np.asarray(x, dtype=np.float32)
    proj_w = np.asarray(proj_w, dtype=np.float32)
    proj_b = np.asarray(proj_b, dtype=np.float32)

    mmnp = mybir.dt.np(MM_DT)
    # xtb[b, p, cc*N + t] = x[b, t, cc*128 + p]; the softmax scale is folded
    # into the query columns (t < LZ) so S arrives pre-scaled
    xtb = x.reshape(B, N, 6, 128).transpose(0, 3, 2, 1).reshape(B, 128, 6 * N)
    xtb = np.ascontiguousarray(xtb, dtype=np.float32).reshape(B, 128, 6, N)
    xtb[:, :, :, :LZ] *= SCALE
    xtb = np.ascontiguousarray(xtb.reshape(B, 128, 6 * N), dtype=mmnp)
    # knb[b, p, j*C + c] = x[b, LZ + j*128 + p, c]
    knb = np.ascontiguousarray(
        x[:, LZ:, :].reshape(B, 2, 128, C).transpose(0, 2, 1, 3).reshape(B, 128, 2 * C),
        dtype=mmnp,
    )
    # pwtb[p, cc*C + m] = proj_w.T[cc*128 + p, m] = proj_w[m, cc*128 + p]
    pwtb = np.ascontiguousarray(
        proj_w.T.reshape(6, 128, C).transpose(1, 0, 2).reshape(128, 6 * C),
        dtype=mmnp,
    )
    b64 = np.ascontiguousarray(np.broadcast_to(proj_b, (128, C)))
    return x, xtb, knb, pwtb, b64


def _run(x, proj_w, proj_b, **spmd_kwargs):
    x, xtb, knb, pwtb, b64 = _host_prep(x, proj_w, proj_b)

    nc = _get_nc()
    in_maps = [
        {
            "xtb": xtb[i * BS : (i + 1) * BS],
            "knb": knb[i * BS : (i + 1) * BS],
            "pwtb": pwtb,
            "bias64": b64,
        }
        for i in range(NCORES)
    ]
    res = run_bass_kernel_spmd(
        nc, in_maps, core_ids=list(range(NCORES)), **spmd_kwargs
    )

    out = np.empty((B, N, C), dtype=np.float32)
    out[:, LZ:, :] = x[:, LZ:, :]
    for i in range(NCORES):
        out[i * BS : (i + 1) * BS, :LZ, :] = res.results[i]["y"].reshape(BS, LZ, C)
    return out, res


def kernel(x, proj_w, proj_b):
    out, _ = _run(x, proj_w, proj_b)
    return out
